# revision 1
# baseline (speedup 1.0000x reference)
"""Trainium2 Bass kernel for nn_NormLearningEngine.

Data-parallel over 8 NeuronCores: batch 64 -> 8 batches per core.
Per core the dominant work is action = x.mean(axis=1) over a 64 MB shard,
done as a ones-vector matmul reduction streamed through SBUF. The small
MLP tail (context encoder, norm selector/matcher, severity head) runs in
"activations-as-columns" orientation so weight matrices are used in their
natural [K, M] layout as lhsT and no on-device weight transposes are
needed. 1-D vectors (biases, nm_w2/sv_w2 columns, constants, identity
blocks) are packed host-side into a single [128, VCOLS] tensor.
"""

import sys

sys.path.insert(0, "/opt/trn_rl_repo")

import numpy as np

import concourse.bacc as bacc
import concourse.tile as tile
from concourse import mybir
from concourse.bass_utils import run_bass_kernel_spmd

F32 = mybir.dt.float32
AF = mybir.ActivationFunctionType
ALU = mybir.AluOpType
AX = mybir.AxisListType

D, H, K, CTXW, T = 1024, 256, 64, 16, 2048
B, NCORES = 64, 8
BPC = B // NCORES  # 8 batches per core
ALPHA = 0.1
EPS = 1e-6

# vpack column map ([128, VCOLS] fp32)
C_ONES = 0       # all ones (col)
C_EPS = 1        # eps everywhere
C_RMSW = 2       # 8 cols: rms_w as columns
C_CEB1 = 10      # 2 cols
C_CEB2 = 12      # 8 cols
C_NMB1 = 20      # 4 cols
C_NSB1 = 24      # 2 cols
C_SVB1 = 26      # 2 cols
C_NMW2 = 28      # 4 cols
C_SVW2 = 32      # 2 cols
C_NSB2 = 34      # 1 col (rows 0:64)
C_NMB2 = 35      # 1 col (row 0)
C_SVB2 = 36      # 1 col (row 0)
C_EYE8 = 40      # 8 cols (rows 0:8 = eye(8))
C_EYE64 = 48     # 64 cols (rows 0:64 = eye(64))
C_ONESROW = 112  # 128 cols of ones (used as a [1,128] row)
VCOLS = 240

# out vector layout (per core, [32])
O_NP, O_WC, O_VIOL, O_SEV = 0, 8, 16, 24


def build_program():
    nc = bacc.Bacc()

    x_d = nc.dram_tensor("x", [BPC, T, D], F32, kind="ExternalInput")
    cb_d = nc.dram_tensor("cb", [CTXW, D], F32, kind="ExternalInput")
    vp_d = nc.dram_tensor("vpack", [128, VCOLS], F32, kind="ExternalInput")
    w1_d = nc.dram_tensor("w1", [D, H], F32, kind="ExternalInput")       # ce_w1
    w2_d = nc.dram_tensor("w2", [H, D], F32, kind="ExternalInput")       # ce_w2
    ns1_d = nc.dram_tensor("ns1", [D, H], F32, kind="ExternalInput")
    ns2_d = nc.dram_tensor("ns2", [H, K], F32, kind="ExternalInput")
    sv1_d = nc.dram_tensor("sv1", [2 * D, H], F32, kind="ExternalInput")
    nm1_d = nc.dram_tensor("nm1", [3 * D, 2 * H], F32, kind="ExternalInput")
    pT_d = nc.dram_tensor("protT", [D, K], F32, kind="ExternalInput")    # protos.T
    out_d = nc.dram_tensor("out", [32], F32, kind="ExternalOutput")

    mm = nc.tensor.matmul

    with tile.TileContext(nc) as tc:
        with (
            tc.tile_pool(name="const", bufs=1) as cp,
            tc.tile_pool(name="xin", bufs=2) as xp,
            tc.tile_pool(name="work", bufs=2) as wk,
            tc.tile_pool(name="ps_t", bufs=3, space="PSUM") as pt,
        ):
            # ---- constant / weight loads (one DMA each) ----
            vp = cp.tile([128, VCOLS], F32)
            nc.sync.dma_start(out=vp[:], in_=vp_d[:])
            cb = cp.tile([CTXW, D], F32)
            nc.sync.dma_start(out=cb[:], in_=cb_d[:])
            w1 = cp.tile([128, D // 128, H], F32)
            nc.sync.dma_start(out=w1[:], in_=w1_d[:].rearrange("(c p) m -> p c m", p=128))
            w2 = cp.tile([128, H // 128, D], F32)
            nc.sync.dma_start(out=w2[:], in_=w2_d[:].rearrange("(c p) m -> p c m", p=128))
            ns1 = cp.tile([128, D // 128, H], F32)
            nc.sync.dma_start(out=ns1[:], in_=ns1_d[:].rearrange("(c p) m -> p c m", p=128))
            ns2 = cp.tile([128, H // 128, K], F32)
            nc.sync.dma_start(out=ns2[:], in_=ns2_d[:].rearrange("(c p) m -> p c m", p=128))
            sv1 = cp.tile([128, 2 * D // 128, H], F32)
            nc.sync.dma_start(out=sv1[:], in_=sv1_d[:].rearrange("(c p) m -> p c m", p=128))
            nm1 = cp.tile([128, 3 * D // 128, 2 * H], F32)
            nc.sync.dma_start(out=nm1[:], in_=nm1_d[:].rearrange("(c p) m -> p c m", p=128))
            ptT = cp.tile([128, D // 128, K], F32)
            nc.sync.dma_start(out=ptT[:], in_=pT_d[:].rearrange("(c p) k -> p c k", p=128))

            ones_col = vp[:, C_ONES : C_ONES + 1]          # [128, 1]
            one11 = vp[0:1, C_ONES : C_ONES + 1]           # [1, 1]
            eye8 = vp[0:8, C_EYE8 : C_EYE8 + 8]            # [8, 8]
            eye64 = vp[0:64, C_EYE64 : C_EYE64 + 64]       # [64, 64]
            ones_row = vp[0:1, C_ONESROW : C_ONESROW + 128]  # [1, 128]

            # =========== T0: context-only pipeline (overlaps x streaming) ===========
            # ctxT[:, c] = (1/16) * sum_t cb[t, c*128:(c+1)*128]   -> [128, 8]
            ctx_ps = pt.tile([128, 8], F32, tag="t")
            for c in range(8):
                mm(out=ctx_ps[:, c : c + 1], lhsT=cb[:, c * 128 : (c + 1) * 128],
                   rhs=vp[0:CTXW, C_ONES : C_ONES + 1], start=True, stop=True)
            ctxT = cp.tile([128, 8], F32)
            nc.scalar.mul(out=ctxT[:], in_=ctx_ps[:], mul=1.0 / CTXW)

            # ce layer 1: h1 = gelu(ce_w1.T @ ctx + ce_b1)  -> [128, 2]
            h1_ps = pt.tile([128, 2], F32, tag="t")
            for m in range(2):
                for c in range(8):
                    mm(out=h1_ps[:, m : m + 1],
                       lhsT=w1[:, c, m * 128 : (m + 1) * 128],
                       rhs=ctxT[:, c : c + 1], start=(c == 0), stop=(c == 7))
            h1 = cp.tile([128, 2], F32)
            for m in range(2):
                nc.scalar.activation(out=h1[:, m : m + 1], in_=h1_ps[:, m : m + 1],
                                     func=AF.Gelu, bias=vp[:, C_CEB1 + m : C_CEB1 + m + 1])

            # ce layer 2: ctx_e = ce_w2.T @ h1 + ce_b2  -> [128, 8]
            ce_ps = pt.tile([128, 8], F32, tag="t")
            for m in range(8):
                for c in range(2):
                    mm(out=ce_ps[:, m : m + 1],
                       lhsT=w2[:, c, m * 128 : (m + 1) * 128],
                       rhs=h1[:, c : c + 1], start=(c == 0), stop=(c == 1))
            ctx_e = cp.tile([128, 8], F32)
            nc.vector.tensor_tensor(out=ctx_e[:], in0=ce_ps[:],
                                    in1=vp[:, C_CEB2 : C_CEB2 + 8], op=ALU.add)

            # rms norm: rstd = 1/sqrt(mean(ctx_e^2) + eps); ctx_enc = ctx_e*rms_w*rstd
            sq = cp.tile([128, 8], F32)
            sqsum = cp.tile([128, 1], F32)
            nc.scalar.activation(out=sq[:], in_=ctx_e[:], func=AF.Square,
                                 accum_out=sqsum[:])
            ms_ps = pt.tile([1, 1], F32, tag="t")
            mm(out=ms_ps[:], lhsT=sqsum[:], rhs=ones_col, start=True, stop=True)
            # x = ms/D + eps (exact), r0 = 1/sqrt via table, then 2 Newton steps
            xms = cp.tile([1, 1], F32)
            nc.vector.tensor_scalar(out=xms[:], in0=ms_ps[:], scalar1=1.0 / D,
                                    scalar2=EPS, op0=ALU.mult, op1=ALU.add)
            sd = cp.tile([1, 1], F32)
            nc.scalar.activation(out=sd[:], in_=ms_ps[:], func=AF.Sqrt,
                                 bias=vp[0:1, C_EPS : C_EPS + 1], scale=1.0 / D)
            r = cp.tile([1, 1], F32)
            nc.vector.reciprocal(out=r[:], in_=sd[:])
            tmp1 = cp.tile([1, 1], F32)
            for _ in range(2):  # r <- r*(1.5 - 0.5*x*r^2)
                nc.vector.tensor_tensor(out=tmp1[:], in0=r[:], in1=r[:], op=ALU.mult)
                nc.vector.tensor_tensor(out=tmp1[:], in0=tmp1[:], in1=xms[:], op=ALU.mult)
                nc.vector.tensor_scalar(out=tmp1[:], in0=tmp1[:], scalar1=-0.5,
                                        scalar2=1.5, op0=ALU.mult, op1=ALU.add)
                nc.vector.tensor_tensor(out=r[:], in0=r[:], in1=tmp1[:], op=ALU.mult)
            # broadcast rstd to a [128, 1] column: ones_row.T @ r
            rb_ps = pt.tile([128, 1], F32, tag="t")
            mm(out=rb_ps[:], lhsT=ones_row, rhs=r[:], start=True, stop=True)
            rb = cp.tile([128, 1], F32)
            nc.vector.tensor_copy(out=rb[:], in_=rb_ps[:])
            ctx_enc = cp.tile([128, 8], F32)
            nc.vector.tensor_tensor(out=ctx_enc[:], in0=ctx_e[:],
                                    in1=vp[:, C_RMSW : C_RMSW + 8], op=ALU.mult)
            nc.vector.tensor_scalar_mul(out=ctx_enc[:], in0=ctx_enc[:], scalar1=rb[:])

            # norm selector: s1 = gelu(ns_w1.T @ ctx_enc + ns_b1) -> [128, 2]
            s1_ps = pt.tile([128, 2], F32, tag="t")
            for m in range(2):
                for c in range(8):
                    mm(out=s1_ps[:, m : m + 1],
                       lhsT=ns1[:, c, m * 128 : (m + 1) * 128],
                       rhs=ctx_enc[:, c : c + 1], start=(c == 0), stop=(c == 7))
            s1 = cp.tile([128, 2], F32)
            for m in range(2):
                nc.scalar.activation(out=s1[:, m : m + 1], in_=s1_ps[:, m : m + 1],
                                     func=AF.Gelu, bias=vp[:, C_NSB1 + m : C_NSB1 + m + 1])
            # logits = ns_w2.T @ s1 + ns_b2 -> [64, 1] column
            lg_ps = pt.tile([64, 1], F32, tag="t")
            for c in range(2):
                mm(out=lg_ps[:], lhsT=ns2[:, c, :], rhs=s1[:, c : c + 1],
                   start=(c == 0), stop=(c == 1))
            lgc = cp.tile([64, 1], F32)
            nc.vector.tensor_scalar_add(out=lgc[:], in0=lg_ps[:],
                                        scalar1=vp[0:64, C_NSB2 : C_NSB2 + 1])
            # transpose to row via eye64, softmax
            lr_ps = pt.tile([1, 64], F32, tag="t")
            mm(out=lr_ps[:], lhsT=lgc[:], rhs=eye64, start=True, stop=True)
            lrow = cp.tile([1, 64], F32)
            nc.vector.tensor_copy(out=lrow[:], in_=lr_ps[:])
            mx = cp.tile([1, 1], F32)
            nc.vector.tensor_reduce(out=mx[:], in_=lrow[:], axis=AX.X, op=ALU.max)
            nmx = cp.tile([1, 1], F32)
            nc.vector.tensor_scalar_mul(out=nmx[:], in0=mx[:], scalar1=-1.0)
            ex = cp.tile([1, 64], F32)
            exsum = cp.tile([1, 1], F32)
            nc.scalar.activation(out=ex[:], in_=lrow[:], func=AF.Exp,
                                 bias=nmx[:], accum_out=exsum[:])
            rexs = cp.tile([1, 1], F32)
            nc.vector.reciprocal(out=rexs[:], in_=exsum[:])
            nw = cp.tile([1, 64], F32)
            nc.vector.tensor_scalar_mul(out=nw[:], in0=ex[:], scalar1=rexs[:])
            nw8 = cp.tile([1, 8 * K], F32)
            for b in range(BPC):
                nc.vector.tensor_copy(out=nw8[:, b * K : (b + 1) * K], in_=nw[:])

            # nm ctx part: u = wc.T @ ctx_enc + nm_b1 -> [128, 4]
            u_ps = pt.tile([128, 4], F32, tag="t")
            for hc in range(4):
                for c in range(8):
                    mm(out=u_ps[:, hc : hc + 1],
                       lhsT=nm1[:, c, hc * 128 : (hc + 1) * 128],
                       rhs=ctx_enc[:, c : c + 1], start=(c == 0), stop=(c == 7))
            u = cp.tile([128, 4], F32)
            nc.vector.tensor_tensor(out=u[:], in0=u_ps[:],
                                    in1=vp[:, C_NMB1 : C_NMB1 + 4], op=ALU.add)

            # nm proto part: PT = wp.T @ protosT -> [128, 4*64]
            pt_ps = pt.tile([128, 4 * K], F32, tag="t")
            for hc in range(4):
                for c in range(8):
                    mm(out=pt_ps[:, hc * K : (hc + 1) * K],
                       lhsT=nm1[:, 16 + c, hc * 128 : (hc + 1) * 128],
                       rhs=ptT[:, c, :], start=(c == 0), stop=(c == 7))
            PTs = cp.tile([128, 4 * K], F32)
            nc.vector.tensor_copy(out=PTs[:], in_=pt_ps[:])

            # severity ctx part: svu = sv_w1[:D].T @ ctx_enc + sv_b1 -> [128, 2]
            svu_ps = pt.tile([128, 2], F32, tag="t")
            for m in range(2):
                for c in range(8):
                    mm(out=svu_ps[:, m : m + 1],
                       lhsT=sv1[:, c, m * 128 : (m + 1) * 128],
                       rhs=ctx_enc[:, c : c + 1], start=(c == 0), stop=(c == 7))
            svu = cp.tile([128, 2], F32)
            nc.vector.tensor_tensor(out=svu[:], in0=svu_ps[:],
                                    in1=vp[:, C_SVB1 : C_SVB1 + 2], op=ALU.add)

            # =========== main stream: action[b] = mean_t x[b] ===========
            # Stage 1 on DVE: per half-batch tile [128(t-part), 8(s), 1024(d)],
            # reduce over s via a strided AP -> [128, 1024]. Stage 2 on PE:
            # per 128-d chunk, V_chunk.T @ ones reduces the 128 t-partitions
            # and lands directly as an actionT column (no transposes needed).
            x_v = x_d[:].rearrange("b (th p s) d -> b th p s d", p=128, s=8)
            aT_ps = pt.tile([128, 8 * BPC], F32, tag="t")

            def slab_add(eng, xt, i, j):
                eng.tensor_tensor(out=xt[:, i, :], in0=xt[:, i, :],
                                  in1=xt[:, j, :], op=ALU.add)

            for b in range(BPC):
                for th in range(2):
                    xt = xp.tile([128, 8, D], F32, tag="xt")
                    nc.sync.dma_start(out=xt[:], in_=x_v[b, th])
                    # in-place pairwise tree: slab 0 ends up with the half-sum
                    slab_add(nc.vector, xt, 0, 1)
                    slab_add(nc.vector, xt, 2, 3)
                    slab_add(nc.vector, xt, 4, 5)
                    slab_add(nc.vector, xt, 6, 7)
                    slab_add(nc.vector, xt, 0, 2)
                    slab_add(nc.vector, xt, 4, 6)
                    slab_add(nc.vector, xt, 0, 4)
                    # stage 2 on PE: chunk.T @ ones accumulates the 128
                    # t-partitions straight into actionT column c*8+b,
                    # accumulating the two halves in PSUM.
                    for c in range(8):
                        mm(out=aT_ps[:, c * BPC + b : c * BPC + b + 1],
                           lhsT=xt[:, 0, c * 128 : (c + 1) * 128],
                           rhs=ones_col, start=(th == 0), stop=(th == 1))
            actionT = cp.tile([128, 8 * BPC], F32)
            nc.vector.tensor_scalar_mul(out=actionT[:], in0=aT_ps[:],
                                        scalar1=1.0 / T)

            # =========== T1: batched tail ===========
            # nm action part: base = wa.T @ actionT -> [128, 4*8]
            base_ps = pt.tile([128, 4 * BPC], F32, tag="t")
            for hc in range(4):
                for c in range(8):
                    mm(out=base_ps[:, hc * BPC : (hc + 1) * BPC],
                       lhsT=nm1[:, 8 + c, hc * 128 : (hc + 1) * 128],
                       rhs=actionT[:, c * BPC : (c + 1) * BPC],
                       start=(c == 0), stop=(c == 7))
            ub = cp.tile([128, 4 * BPC], F32)
            for hc in range(4):
                nc.vector.tensor_scalar_add(out=ub[:, hc * BPC : (hc + 1) * BPC],
                                            in0=base_ps[:, hc * BPC : (hc + 1) * BPC],
                                            scalar1=u[:, hc : hc + 1])

            # conformance logits: for each b: sum_h nm_w2[h]*gelu(PT + ub)[h, k]
            conf_ps = pt.tile([1, BPC * K], F32, tag="t")
            for b in range(BPC):
                pre = wk.tile([128, 4 * K], F32, tag="pre")
                for hc in range(4):
                    nc.vector.tensor_scalar_add(out=pre[:, hc * K : (hc + 1) * K],
                                                in0=PTs[:, hc * K : (hc + 1) * K],
                                                scalar1=ub[:, hc * BPC + b : hc * BPC + b + 1])
                g = wk.tile([128, 4 * K], F32, tag="g")
                nc.scalar.activation(out=g[:], in_=pre[:], func=AF.Gelu)
                for hc in range(4):
                    mm(out=conf_ps[0:1, b * K : (b + 1) * K],
                       lhsT=vp[:, C_NMW2 + hc : C_NMW2 + hc + 1],
                       rhs=g[:, hc * K : (hc + 1) * K],
                       start=(hc == 0), stop=(hc == 3))
            conf = cp.tile([1, BPC * K], F32)
            nc.scalar.activation(out=conf[:], in_=conf_ps[:], func=AF.Sigmoid,
                                 bias=vp[0:1, C_NMB2 : C_NMB2 + 1])

            out_sb = cp.tile([1, 32], F32)
            # weighted_conf[b] = sum_k conf[b, k] * nw[k]
            prod = cp.tile([1, BPC * K], F32)
            nc.vector.tensor_tensor(out=prod[:], in0=conf[:], in1=nw8[:], op=ALU.mult)
            nc.vector.tensor_reduce(out=out_sb[0:1, O_WC : O_WC + 8],
                                    in_=prod[:].rearrange("p (b k) -> p b k", b=BPC),
                                    axis=AX.X, op=ALU.add)
            # violation = 1 - weighted_conf
            nc.vector.tensor_scalar(out=out_sb[0:1, O_VIOL : O_VIOL + 8],
                                    in0=out_sb[0:1, O_WC : O_WC + 8],
                                    scalar1=-1.0, scalar2=1.0, op0=ALU.mult, op1=ALU.add)

            # severity: sv = sigmoid(sv_w2.T @ gelu(sv_w1[D:].T @ actionT + svu) + sv_b2)
            sv_ps = pt.tile([128, 2 * BPC], F32, tag="t")
            for m in range(2):
                for c in range(8):
                    mm(out=sv_ps[:, m * BPC : (m + 1) * BPC],
                       lhsT=sv1[:, 8 + c, m * 128 : (m + 1) * 128],
                       rhs=actionT[:, c * BPC : (c + 1) * BPC],
                       start=(c == 0), stop=(c == 7))
            svg = cp.tile([128, 2 * BPC], F32)
            for m in range(2):
                nc.scalar.activation(out=svg[:, m * BPC : (m + 1) * BPC],
                                     in_=sv_ps[:, m * BPC : (m + 1) * BPC],
                                     func=AF.Gelu, bias=svu[:, m : m + 1])
            sev_ps = pt.tile([1, BPC], F32, tag="t")
            for m in range(2):
                mm(out=sev_ps[:], lhsT=vp[:, C_SVW2 + m : C_SVW2 + m + 1],
                   rhs=svg[:, m * BPC : (m + 1) * BPC], start=(m == 0), stop=(m == 1))
            nc.scalar.activation(out=out_sb[0:1, O_SEV : O_SEV + 8], in_=sev_ps[:],
                                 func=AF.Sigmoid, bias=vp[0:1, C_SVB2 : C_SVB2 + 1])

            # norm_penalty = alpha * violation * severity
            nc.vector.tensor_tensor(out=out_sb[0:1, O_NP : O_NP + 8],
                                    in0=out_sb[0:1, O_VIOL : O_VIOL + 8],
                                    in1=out_sb[0:1, O_SEV : O_SEV + 8], op=ALU.mult)
            nc.vector.tensor_scalar_mul(out=out_sb[0:1, O_NP : O_NP + 8],
                                        in0=out_sb[0:1, O_NP : O_NP + 8], scalar1=ALPHA)

            nc.sync.dma_start(out=out_d[:].rearrange("(p n) -> p n", p=1),
                              in_=out_sb[0:1, :])

    nc.finalize()
    return nc


def _build_vpack(inp):
    vp = np.zeros((128, VCOLS), np.float32)

    def cols(v, c0):
        v = np.asarray(v, np.float32).reshape(-1)
        ncols = (len(v) + 127) // 128
        for c in range(ncols):
            seg = v[c * 128 : (c + 1) * 128]
            vp[: len(seg), c0 + c] = seg

    vp[:, C_ONES] = 1.0
    vp[:, C_EPS] = EPS
    cols(inp["rms_w"], C_RMSW)
    cols(inp["ce_b1"], C_CEB1)
    cols(inp["ce_b2"], C_CEB2)
    cols(inp["nm_b1"], C_NMB1)
    cols(inp["ns_b1"], C_NSB1)
    cols(inp["sv_b1"], C_SVB1)
    cols(inp["nm_w2"], C_NMW2)
    cols(inp["sv_w2"], C_SVW2)
    cols(inp["ns_b2"], C_NSB2)
    cols(inp["nm_b2"], C_NMB2)
    cols(inp["sv_b2"], C_SVB2)
    vp[0:8, C_EYE8 : C_EYE8 + 8] = np.eye(8, dtype=np.float32)
    vp[0:64, C_EYE64 : C_EYE64 + 64] = np.eye(64, dtype=np.float32)
    vp[0, C_ONESROW : C_ONESROW + 128] = 1.0
    return vp


_CACHE = {}


def _in_maps(inputs):
    f = lambda k: np.ascontiguousarray(np.asarray(inputs[k], np.float32))
    x = f("x")
    cb = f("context_buffer").reshape(CTXW, D)
    vp = _build_vpack({k: np.asarray(v) for k, v in inputs.items()})
    protT = np.ascontiguousarray(np.asarray(inputs["norm_prototypes"], np.float32).T)
    shared = {
        "cb": cb, "vpack": vp,
        "w1": f("ce_w1"), "w2": f("ce_w2"),
        "ns1": f("ns_w1"), "ns2": f("ns_w2"),
        "sv1": f("sv_w1"), "nm1": f("nm_w1"),
        "protT": protT,
    }
    return [dict(shared, x=np.ascontiguousarray(x[c * BPC : (c + 1) * BPC]))
            for c in range(NCORES)]


def run(inputs, trace=False, tmpdir=None):
    if "nc" not in _CACHE:
        _CACHE["nc"] = build_program()
    res = run_bass_kernel_spmd(_CACHE["nc"], _in_maps(inputs),
                               list(range(NCORES)), trace=trace, tmpdir=tmpdir)
    npen = np.empty(B, np.float32)
    wc = np.empty(B, np.float32)
    viol = np.empty(B, np.float32)
    sev = np.empty(B, np.float32)
    for c in range(NCORES):
        o = res.results[c]["out"]
        npen[c * BPC : (c + 1) * BPC] = o[O_NP : O_NP + 8]
        wc[c * BPC : (c + 1) * BPC] = o[O_WC : O_WC + 8]
        viol[c * BPC : (c + 1) * BPC] = o[O_VIOL : O_VIOL + 8]
        sev[c * BPC : (c + 1) * BPC] = o[O_SEV : O_SEV + 8]
    return (npen, wc, viol, sev), res


def kernel(**inputs):
    outs, _ = run(inputs, trace=False)
    return outs

